# revision 60
# baseline (speedup 1.0000x reference)
"""Trainium2 Bass kernel for nn_MultiHeadAttention (B=4, S=2048, D=512, H=8).

Sharding: 8 cores = 4 batches x 2 head-groups (4 heads each).
Each core computes, for its (b, hg):
    Q/K/V projections (its 4 heads) -> masked softmax attention -> partial
    output projection  partial_hg = x_hg @ Wo[:, hg_cols].T   (row-sharded).
Host side: inputs are pre-transposed/sliced per core; outputs are summed
across the 2 head-groups per batch and bo is added.

Device layouts (per core), S=2048, D=512, FPC=256 (features per core):
  qT/kT/vT  [D, S]   f32  (host-transposed activations)
  keepT     [S, S]   bf16 (1.0 where attention allowed, 0.0 where masked; [k, q])
  wqT/wkT/wvT [D, FPC] f32 ; woT [FPC, D] f32 (host-transposed weight slices)
  bq/bk [128, 2] f32 ; bv [64, 4] f32  (host-shaped per-partition biases)
  outT      [D, S]   f32  (partial output, transposed)
"""

import sys

if "/opt/trn_rl_repo" not in sys.path:
    sys.path.insert(0, "/opt/trn_rl_repo")

import numpy as np
import ml_dtypes

import concourse.bass as bass
import concourse.mybir as mybir
import concourse.tile as tile
from concourse import bacc
from concourse import bass_utils

# Steer Exp/Ln to the combined table set so the kernel never reloads ACT
# tables: hide Exp/Ln from every other set (set ids / insertion order are
# preserved, so act_func_set_id stays consistent with act_info.json).
_orig_get_tables = bacc.get_activation_tables


def _patched_get_tables(arch):
    tables = {k: set(v) for k, v in _orig_get_tables(arch).items()}
    for name, fns in tables.items():
        if name != "natural_log_exp_and_others":
            fns.discard(mybir.ActivationFunctionType.Exp)
            fns.discard(mybir.ActivationFunctionType.Ln)
    return tables


bacc.get_activation_tables = _patched_get_tables

F32 = mybir.dt.float32
F32R = mybir.dt.float32r
BF16 = mybir.dt.bfloat16
EXP = mybir.ActivationFunctionType.Exp
LOG = mybir.ActivationFunctionType.Ln
MULT = mybir.AluOpType.mult

B = 4
S = 2048
D = 512
H = 8
DK = 64
HPC = 4          # heads per core
FPC = HPC * DK   # 256 projected features per core
NEG_BIG = -81920.0  # unused on-device (mask is multiplicative) but kept for reference


def build_kernel(s=S, d=D, debug_taps=False):
    """Build and compile the per-core Bass program. Returns compiled nc."""
    n_kt = s // 128          # 128-row k tiles
    n_qc = s // 512          # 512-col q chunks
    n_dc = d // 128          # 128-row d chunks
    n_ft = FPC // 128        # 128-row feature tiles (2)
    half = n_kt // 2

    nc = bacc.Bacc(
        "TRN2",
        target_bir_lowering=False,
        debug=False,
        enable_asserts=False,
        num_devices=8,
    )

    qT = nc.dram_tensor("qT", [d, s], F32R, kind="ExternalInput").ap()
    kT = nc.dram_tensor("kT", [d, s], F32R, kind="ExternalInput").ap()
    vT = nc.dram_tensor("vT", [d, s], F32R, kind="ExternalInput").ap()
    keepT = nc.dram_tensor("keepT", [s, s], BF16, kind="ExternalInput").ap()
    wqT = nc.dram_tensor("wqT", [d, FPC], F32R, kind="ExternalInput").ap()
    wkT = nc.dram_tensor("wkT", [d, FPC], F32R, kind="ExternalInput").ap()
    wvT = nc.dram_tensor("wvT", [d, FPC], F32R, kind="ExternalInput").ap()
    woT = nc.dram_tensor("woT", [FPC, d], F32R, kind="ExternalInput").ap()
    bq = nc.dram_tensor("bq", [128, n_ft], F32, kind="ExternalInput").ap()
    bk = nc.dram_tensor("bk", [128, n_ft], F32, kind="ExternalInput").ap()
    bv = nc.dram_tensor("bv", [64, HPC], F32, kind="ExternalInput").ap()
    outT = nc.dram_tensor("outT", [d, s], F32, kind="ExternalOutput").ap()

    kT_r = kT.rearrange("(c p) s -> p c s", p=128)
    qT_r = qT.rearrange("(c p) s -> p c s", p=128)
    vT_r = vT.rearrange("(c p) s -> p c s", p=128)
    keepT_r = keepT.rearrange("(t p) q -> p t q", p=128)

    with tile.TileContext(nc) as tc:
        with (
            tc.tile_pool(name="weights", bufs=1) as wpool,
            tc.tile_pool(name="resident", bufs=1) as rpool,
            tc.tile_pool(name="stage", bufs=4) as stpool,
            tc.tile_pool(name="vstage", bufs=3) as vpool,
            tc.tile_pool(name="keeppool", bufs=2) as keeppool,
            tc.tile_pool(name="epool", bufs=4) as epool,
            tc.tile_pool(name="xpool", bufs=2) as xpool,
            tc.tile_pool(name="zpool", bufs=1) as zpool,
            tc.tile_pool(name="zdram", bufs=2, space="DRAM") as zdram,
            tc.tile_pool(name="outpool", bufs=2) as outpool,
            tc.tile_pool(name="spsum", bufs=2, space="PSUM") as spsum,
            tc.tile_pool(name="opsum", bufs=2, space="PSUM") as opsum,
        ):
            # ---- resident tensors ----
            QT_t = rpool.tile([128, n_ft, s], F32R, tag="QT")
            KT_t = rpool.tile([128, n_ft, s], F32R, tag="KT")
            # Vaug: [128 (k within tile), n_kt, HPC*65]; per head 64 V cols + ones.
            Vaug_t = rpool.tile([128, n_kt, HPC * 65], BF16, tag="Vaug")

            wq_t = wpool.tile([128, n_dc, FPC], F32R, tag="wq")
            wk_t = wpool.tile([128, n_dc, FPC], F32R, tag="wk")
            wv_t = wpool.tile([128, n_dc, FPC], F32R, tag="wv")
            wo_t = wpool.tile([64, HPC, d], F32R, tag="wo")
            bq_t = wpool.tile([128, n_ft], F32, tag="bq")
            bk_t = wpool.tile([128, n_ft], F32, tag="bk")
            bv_t = wpool.tile([64, HPC], F32, tag="bv")

            ones_ap = Vaug_t.rearrange("p t (h c) -> p t h c", c=65)[:, :, :, 64:65]
            nc.vector.memset(ones_ap, 1.0)

            # ---- DMAs in order of first use: K path first ----
            nc.sync.dma_start(wk_t[:], wkT.rearrange("(c p) f -> p c f", p=128))
            nc.sync.dma_start(bk_t[:], bk[:])
            k_st = [stpool.tile([128, n_dc, 512], F32R, tag="stage", name="k_st0")]
            # dc-chunked so the first projection matmul starts ~2us earlier
            for dc in range(n_dc):
                nc.sync.dma_start(k_st[0][:, dc, :], kT_r[:, dc, 0:512])
            nc.sync.dma_start(wq_t[:], wqT.rearrange("(c p) f -> p c f", p=128))
            nc.sync.dma_start(bq_t[:], bq[:])
            q_st = [None] * (s // 512)
            q_st[0] = stpool.tile([128, n_dc, 512], F32R, tag="stage", name="q_st0")
            for dc in range(n_dc):
                nc.sync.dma_start(q_st[0][:, dc, :], qT_r[:, dc, 0:512])
            for sc in range(1, s // 512):
                t = stpool.tile([128, n_dc, 512], F32R, tag="stage", name=f"k_st{sc}")
                nc.sync.dma_start(t[:], kT_r[:, :, sc * 512 : (sc + 1) * 512])
                k_st.append(t)
            def load_keep(qc):
                halves = []
                for hf in range(2):
                    t = keeppool.tile([128, half, 512], BF16, tag="keep")
                    nc.sync.dma_start(
                        t[:],
                        keepT_r[:, hf * half : (hf + 1) * half,
                                qc * 512 : (qc + 1) * 512],
                    )
                    halves.append(t)
                return halves

            nc.sync.dma_start(wv_t[:], wvT.rearrange("(c p) f -> p c f", p=128))
            nc.sync.dma_start(bv_t[:], bv[:])

            # ---- projection helpers ----
            def emit_qk_proj(x_st_tile, w_t, b_t, dst_t, ft, sc):
                ps = spsum.tile([128, 512], F32, tag="S")
                for dc in range(n_dc):
                    nc.tensor.matmul(
                        ps[:],
                        w_t[:, dc, ft * 128 : (ft + 1) * 128],
                        x_st_tile[:, dc, :],
                        start=(dc == 0),
                        stop=(dc == n_dc - 1),
                    )
                nc.vector.tensor_scalar_add(
                    dst_t[:, ft, sc * 512 : (sc + 1) * 512],
                    ps[:],
                    b_t[:, ft : ft + 1],
                )

            GROUPS = []
            g0 = 0
            while g0 < n_kt:
                g = min(2, n_kt - g0)
                GROUPS.append((g0, g))
                g0 += g

            filler_q = []

            def emit_scores(qc, h, keep_halves, fill=False):
                qlo, qhi = qc * 512, (qc + 1) * 512
                ft = h // 2
                plo = (h % 2) * 64
                phi = plo + 64
                E_t = epool.tile([128, n_kt, 512], BF16, tag="E")
                for kt0, g in GROUPS:
                    sp = spsum.tile([128, 2 * 512], F32, tag="S")
                    for i in range(g):
                        ktile = kt0 + i
                        nc.tensor.matmul(
                            sp[:, i * 512 : (i + 1) * 512],
                            KT_t[plo:phi, ft, ktile * 128 : (ktile + 1) * 128],
                            QT_t[plo:phi, ft, qlo:qhi],
                            start=True,
                            stop=True,
                        )
                    if fill and filler_q:
                        filler_q.pop(0)()
                    nc.scalar.activation(
                        E_t[:, kt0 : kt0 + g, :],
                        sp[:, 0 : g * 512],
                        EXP,
                        scale=0.125,
                    )
                for hf in range(2):
                    nc.vector.tensor_tensor(
                        E_t[:, hf * half : (hf + 1) * half, :],
                        E_t[:, hf * half : (hf + 1) * half, :],
                        keep_halves[hf][:],
                        MULT,
                    )
                return E_t

            def emit_attnv(h, E_t, op):
                for ktile in range(n_kt):
                    nc.tensor.matmul(
                        op[0:65, h % 2, :],
                        Vaug_t[:, ktile, h * 65 : (h + 1) * 65],
                        E_t[:, ktile, :],
                        start=(ktile == 0),
                        stop=(ktile == n_kt - 1),
                    )

            def emit_normalize(pair, op, xT_t):
                # 1/Z = exp(-ln Z) for two heads; broadcast via DRAM bounce
                rz = zpool.tile([65, 2 * 512], F32, tag="rz", name=f"rz_{pair}")
                nc.scalar.activation(rz[64:65, :], op[64:65, :, :], LOG)
                nc.scalar.activation(rz[64:65, :], rz[64:65, :], EXP, scale=-1.0)
                zd = zdram.tile([2 * 512], F32, tag="zd", name=f"zd_{pair}")
                nc.sync.dma_start(zd[:], rz[64:65, :])
                rzb = zpool.tile([64, 2, 512], F32, tag="rzb", name=f"rzb_{pair}")
                nc.sync.dma_start(
                    rzb[:],
                    zd.rearrange("(h q) -> h q", q=512)[None, :, :].to_broadcast(
                        [64, 2, 512]
                    ),
                )
                nc.vector.tensor_tensor(
                    xT_t[:, 2 * pair : 2 * pair + 2, :], op[0:64, :, :], rzb[:], MULT
                )
                for hh in range(2):
                    h = 2 * pair + hh
                    nc.vector.tensor_scalar_add(
                        xT_t[:, h, :], xT_t[:, h, :], bv_t[:, h : h + 1]
                    )

            def emit_outproj_ft(qc, xT_prev, ftile):
                qlo, qhi = qc * 512, (qc + 1) * 512
                po = spsum.tile([128, 512], F32, tag="S", name=f"po_{qc}_{ftile}")
                for h in range(HPC):
                    nc.tensor.matmul(
                        po[:],
                        wo_t[:, h, ftile * 128 : (ftile + 1) * 128],
                        xT_prev[:, h, :],
                        start=(h == 0),
                        stop=(h == HPC - 1),
                    )
                ot = outpool.tile([128, 512], F32, tag="out", name=f"ot_{qc}_{ftile}")
                nc.vector.tensor_copy(ot[:], po[:])
                nc.sync.dma_start(
                    outT[ftile * 128 : (ftile + 1) * 128, qlo:qhi], ot[:]
                )

            def emit_outproj(qc, xT_prev):
                for ftile in range(d // 128):
                    emit_outproj_ft(qc, xT_prev, ftile)

            # ---- K(sc0) + Q(sc0, ft0) now; the rest ride the filler wave ----
            for ft in range(n_ft):
                emit_qk_proj(k_st[0], wk_t, bk_t, KT_t, ft, 0)
            emit_qk_proj(q_st[0], wq_t, bq_t, QT_t, 0, 0)
            # remaining input DMAs (keep queue order = order of first use)
            nc.sync.dma_start(wv_t[:], wvT.rearrange("(c p) f -> p c f", p=128))
            nc.sync.dma_start(bv_t[:], bv[:])
            v_st = []
            for st in range(n_kt):
                vt = vpool.tile([128, n_dc, 128], F32R, tag="vst", name=f"v_st{st}")
                nc.sync.dma_start(vt[:], vT_r[:, :, st * 128 : (st + 1) * 128])
                v_st.append(vt)
            keep_t = load_keep(0)
            for sc in range(1, s // 512):
                q_st[sc] = stpool.tile([128, n_dc, 512], F32R, tag="stage", name=f"q_st{sc}")
                nc.sync.dma_start(q_st[sc][:], qT_r[:, :, sc * 512 : (sc + 1) * 512])
            nc.sync.dma_start(wo_t[:], woT.rearrange("(c p) f -> p c f", p=64))

            # V projection / remaining Q projections become scores-gap fillers
            def make_vproj(st):
                def _f():
                    psv = spsum.tile([128, FPC], F32, tag="S", name=f"psv{st}")
                    for dc in range(n_dc):
                        nc.tensor.matmul(
                            psv[:],
                            v_st[st][:, dc, :],
                            wv_t[:, dc, :],
                            start=(dc == 0),
                            stop=(dc == n_dc - 1),
                        )
                    dst = Vaug_t.rearrange("p t (h c) -> p t h c", c=65)[
                        :, st, :, 0:64
                    ]
                    nc.vector.tensor_copy(
                        dst, psv.rearrange("p (h c) -> p h c", c=64)
                    )
                return _f

            def make_qproj(ft, sc):
                return lambda: emit_qk_proj(q_st[sc], wq_t, bq_t, QT_t, ft, sc)

            def make_kproj(ft, sc):
                return lambda: emit_qk_proj(k_st[sc], wk_t, bk_t, KT_t, ft, sc)

            opsA = {}
            opsB = {}
            emitted_attnv = set()
            E_q = {}

            def make_attnv0(h):
                def _f():
                    if 0 not in opsA:
                        opsA[0] = opsum.tile([65, 2, 512], F32, tag="O", name="opA_0")
                    emit_attnv(h, E_q[(0, h)], opsA[0])
                    emitted_attnv.add((0, h))
                return _f

            for sc in range(1, s // 512):
                for ft in range(n_ft):
                    filler_q.append(make_kproj(ft, sc))
            filler_q.append(make_qproj(1, 0))
            for st in range(n_kt):
                filler_q.append(make_vproj(st))
            filler_q.append(make_attnv0(0))
            for sc in range(1, s // 512):
                for ft in range(n_ft):
                    filler_q.append(make_qproj(ft, sc))
            filler_q.append(make_attnv0(1))
            E_q[(0, 0)] = emit_scores(0, 0, keep_t, fill=True)
            E_q[(0, 1)] = emit_scores(0, 1, keep_t, fill=True)
            E_q[(0, 2)] = emit_scores(0, 2, keep_t, fill=True)
            E_q[(0, 3)] = emit_scores(0, 3, keep_t, fill=True)
            while filler_q:
                filler_q.pop(0)()

            # ---- main attention loop: 1-head software pipeline ----
            tasks = [(qc, h) for qc in range(n_qc) for h in range(HPC)]
            keeps = {0: keep_t}
            xTs = {}
            xT_prev = None
            for idx, (qc, h) in enumerate(tasks):
                if h == 0:
                    xTs[qc] = xpool.tile(
                        [64, HPC, 512], F32R, tag="xT", name=f"xT_{qc}"
                    )
                    if qc not in opsA:
                        opsA[qc] = opsum.tile(
                            [65, 2, 512], F32, tag="O", name=f"opA_{qc}"
                        )
                if h == 2:
                    opsB[qc] = opsum.tile(
                        [65, 2, 512], F32, tag="O", name=f"opB_{qc}"
                    )
                def ensure_keep(kqc):
                    if kqc not in keeps:
                        keeps[kqc] = load_keep(kqc)

                if qc == 0 and h >= 1 and n_qc > 1:
                    ensure_keep(1)
                    E_q[(1, h - 1)] = emit_scores(1, h - 1, keeps[1], fill=True)
                # emit the next two tasks' scores ahead of this task's attnV
                for off in (1, 2):
                    if idx + off < len(tasks):
                        nqc, nh = tasks[idx + off]
                        if (nqc, nh) not in E_q:
                            ensure_keep(nqc)
                            E_q[(nqc, nh)] = emit_scores(
                                nqc, nh, keeps[nqc], fill=True
                            )
                if h == 0 and qc > 0:
                    # previous qc's pair-B normalize, queued behind fresh exps
                    emit_normalize(1, opsB[qc - 1], xTs[qc - 1])
                    xT_prev = xTs[qc - 1]
                E_t = E_q.pop((qc, h))
                if (qc, h) not in emitted_attnv:
                    emit_attnv(h, E_t, opsA[qc] if h < 2 else opsB[qc])
                if h == 2:
                    emit_normalize(0, opsA[qc], xTs[qc])
                    if xT_prev is not None:
                        xp = xT_prev
                        oqc = qc - 1
                        for ftile in range(d // 128):
                            filler_q.append(
                                (lambda f=ftile, x=xp, q=oqc:
                                 emit_outproj_ft(q, x, f))
                            )
                    if qc + 2 < n_qc:
                        ensure_keep(qc + 2)
            emit_normalize(1, opsB[n_qc - 1], xTs[n_qc - 1])
            while filler_q:
                filler_q.pop(0)()
            emit_outproj(n_qc - 1, xTs[n_qc - 1])

    nc.compile()
    return nc


_CACHED_NC = None


def _get_nc():
    global _CACHED_NC
    if _CACHED_NC is None:
        _CACHED_NC = build_kernel()
    return _CACHED_NC


def make_in_maps(query, key, value, mask, Wq, bq, Wk, bk, Wv, bv, Wo, bo):
    n_ft = FPC // 128
    in_maps = []
    for c in range(8):
        b, hg = c // 2, c % 2
        lo, hi = hg * FPC, (hg + 1) * FPC
        keep = (~mask[b]).T.astype(ml_dtypes.bfloat16)  # [k, q]
        m = {
            "qT": np.ascontiguousarray(query[b].T, dtype=np.float32),
            "kT": np.ascontiguousarray(key[b].T, dtype=np.float32),
            "vT": np.ascontiguousarray(value[b].T, dtype=np.float32),
            "keepT": np.ascontiguousarray(keep),
            "wqT": np.ascontiguousarray(Wq[lo:hi].T, dtype=np.float32),
            "wkT": np.ascontiguousarray(Wk[lo:hi].T, dtype=np.float32),
            "wvT": np.ascontiguousarray(Wv[lo:hi].T, dtype=np.float32),
            "woT": np.ascontiguousarray(Wo[:, lo:hi].T, dtype=np.float32),
            "bq": np.ascontiguousarray(
                bq[lo:hi].reshape(n_ft, 128).T, dtype=np.float32
            ),
            "bk": np.ascontiguousarray(
                bk[lo:hi].reshape(n_ft, 128).T, dtype=np.float32
            ),
            "bv": np.ascontiguousarray(
                bv[lo:hi].reshape(HPC, 64).T, dtype=np.float32
            ),
        }
        in_maps.append(m)
    return in_maps


def kernel(query, key, value, mask, Wq, bq, Wk, bk, Wv, bv, Wo, bo):
    query = np.asarray(query, dtype=np.float32)
    key = np.asarray(key, dtype=np.float32)
    value = np.asarray(value, dtype=np.float32)
    mask = np.asarray(mask).astype(bool)
    Wq, bq = np.asarray(Wq, np.float32), np.asarray(bq, np.float32)
    Wk, bk = np.asarray(Wk, np.float32), np.asarray(bk, np.float32)
    Wv, bv = np.asarray(Wv, np.float32), np.asarray(bv, np.float32)
    Wo, bo = np.asarray(Wo, np.float32), np.asarray(bo, np.float32)

    nc = _get_nc()
    in_maps = make_in_maps(
        query, key, value, mask, Wq, bq, Wk, bk, Wv, bv, Wo, bo
    )
    res = bass_utils.run_bass_kernel_spmd(nc, in_maps, core_ids=list(range(8)))

    out = np.empty((B, S, D), dtype=np.float32)
    for b in range(B):
        partial = res.results[2 * b]["outT"] + res.results[2 * b + 1]["outT"]
        out[b] = partial.T + bo[None, :]
    return out
